# revision 11
# baseline (speedup 1.0000x reference)
"""Trainium2 Bass kernel for nn_CreateOverlappingWindows.

out[b, t, w*C + c] = x_padded[b, t + w, c]  (SAME zero padding, n_context=9)

Flattening (w, c) -> 494 contiguous values, each output row is a contiguous
494-element window of the zero-padded flattened input:
    out[b, t, :] = xpad_flat[b, t*C : t*C + W*C]

Strategy (memory-regime, bf16 end-to-end):
  * All 4 per-core batches go through SBUF.  128 partitions x 16 rows per
    batch (T padded to 2048 on device, trimmed on host).
  * SBUF AXI ports are the binding resource: port = ((p>>2)&7)<<1|(p>>6),
    27 GB/s each.  A HWDGE InstDMACopy splits its outer dim over SDMA
    engines in contiguous runs (engines = largest divisor <= 16), so a
    64-partition DMA gives each engine exactly one 4-partition port
    group.  Stores are issued as group A (partitions 0-63, even ports)
    on the sync ring CONCURRENT with group B (partitions 64-127, odd
    ports) on the scalar ring - disjoint port halves.  Loads split the
    same way (one sem per batch, incremented +16 by each half -> 32).
  * The 26 -> 494 window expansion runs per chunk of (4, 6, 6) rows:
    the 4-row chunk is DVE-only so the store pipe starts early; the
    6-row chunks split DVE (int32-viewed copies) / ACT (native bf16 -
    ACT's fp path would round int32 views).  A dummy ACT copy on a
    scratch tile preloads the activation table off the critical path.
  * Engine-program order does NOT order a dma_start after an in-flight
    copy: every store is gated on the ev/ea semaphores (true completion).

Sharding: pure data parallel - batch 32 split 4-per-core across 8 cores.
"""

import sys

sys.path.insert(0, "/opt/trn_rl_repo")

import ml_dtypes
import numpy as np
from concourse import bass, mybir
from concourse.ap import AP
from concourse.bass_utils import run_bass_kernel_spmd

_BF16 = mybir.dt.bfloat16
_I32 = mybir.dt.int32
_NPBF16 = ml_dtypes.bfloat16

_NCORES = 8
_B, _T, _C = 32, 2000, 26
_NCTX = 9
_W = 2 * _NCTX + 1  # 19
_WC = _W * _C  # 494
_PAD = _NCTX * _C  # 234
_BPC = _B // _NCORES  # 4 batches per core

_P = 128  # partitions per batch
_R = 16  # output rows per partition
_TV = _P * _R  # 2048 device-side rows (rows 2000+ are discarded on host)
_SEG = _R * _C + (_WC - _C)  # 884: input slice length incl. halo
_NP = (_P - 1) * _R * _C + _SEG  # 53716 padded flat input length per batch
_RW = _R * _WC  # 7904 output elems per partition per batch
_TWC = _TV * _WC  # 1011712 device-side output elems per batch
_FI = _BPC * _SEG  # 3536 free elems/partition, input tile
_FO = _BPC * _RW  # 31616 free elems/partition, output tile

# per batch: 3 chunks of (4, 6, 6) rows; DVE rows per chunk, rest ACT
_CR = (4, 6, 6)
_CR0 = (0, 4, 10)  # start row of each chunk
_DR = (4, 3, 3)  # DVE rows (ACT gets _CR - _DR: 0, 3, 3)
_NCH = len(_CR)
_HG = _P // 2  # 64 partitions per store group

_nc_cache = None


def _build():
    global _nc_cache
    if _nc_cache is not None:
        return _nc_cache
    nc = bass.Bass()
    xp = nc.declare_dram_parameter("xp", [_BPC, _NP], _BF16, isOutput=False)
    out = nc.declare_dram_parameter("out", [_BPC, _TV, _WC], _BF16, isOutput=True)

    with (
        nc.sbuf_tensor([128, _FI], _BF16) as tin,
        nc.sbuf_tensor([128, _FO], _BF16) as tout,
        nc.sbuf_tensor([128, 2], _BF16) as scratch,
        nc.Block() as block,
        nc.semaphore("l0") as l0,
        nc.semaphore("l1") as l1,
        nc.semaphore("l2") as l2,
        nc.semaphore("l3") as l3,
        nc.semaphore("ev") as ev,
        nc.semaphore("ea") as ea,
        nc.semaphore("ss") as ss,
    ):
        lsem = [l0, l1, l2, l3]

        def load_half(e, b, g):
            return e.dma_start(
                out=AP(
                    tin,
                    g * _HG * _FI + b * _SEG,
                    [[_FI, _HG], [1, _SEG]],
                ),
                in_=AP(
                    xp,
                    b * _NP + g * _HG * _R * _C,
                    [[_R * _C, _HG], [1, _SEG]],
                ),
            ).then_inc(lsem[b], 16)

        def store_chunk(e, b, j, g):
            r0, n = _CR0[j], _CR[j] * _WC
            return e.dma_start(
                out=AP(
                    out,
                    b * _TWC + g * _HG * _RW + r0 * _WC,
                    [[_RW, _HG], [1, n]],
                ),
                in_=AP(
                    tout,
                    g * _HG * _FO + b * _RW + r0 * _WC,
                    [[_FO, _HG], [1, n]],
                ),
            ).then_inc(ss, 16)

        def wait_chunk(e, b, j):
            e.wait_ge(ev, _NCH * b + j + 1)
            if j > 0:  # chunk 0 has no ACT rows
                e.wait_ge(ea, 2 * b + j)

        @block.sync
        def _(e):
            for b in range(_BPC):
                load_half(e, b, 0)
            for b in range(_BPC):
                for j in range(_NCH):
                    wait_chunk(e, b, j)
                    store_chunk(e, b, j, 0)  # group A: even ports
            e.wait_ge(ss, 16 * 2 * _NCH * _BPC)

        @block.vector
        def _(v):
            for b in range(_BPC):
                v.wait_ge(lsem[b], 32)
                for j in range(_NCH):
                    r0 = _CR0[j]
                    v.tensor_copy(
                        out=AP(
                            tout,
                            b * _RW + r0 * _WC,
                            [[_FO, _P], [_WC, _DR[j]], [1, _WC]],
                        ).bitcast(_I32),
                        in_=AP(
                            tin,
                            b * _SEG + r0 * _C,
                            [[_FI, _P], [_C, _DR[j]], [1, _WC]],
                        ).bitcast(_I32),
                    ).then_inc(ev, 1)

        @block.scalar
        def _(e):
            for b in range(_BPC):
                load_half(e, b, 1)
            # dummy ACT op: pulls the activation table load off the
            # critical path (first real copy would otherwise pay ~1.3us)
            e.copy(out=AP(scratch, 0, [[2, 1], [1, 2]]),
                   in_=AP(scratch, 0, [[2, 1], [1, 2]]))
            for b in range(_BPC):
                e.wait_ge(ev, _NCH * b + 1)
                store_chunk(e, b, 0, 1)  # chunk 0, group B: DVE-only rows
                e.wait_ge(lsem[b], 32)
                for j in range(1, _NCH):
                    r0 = _CR0[j] + _DR[j]
                    nr = _CR[j] - _DR[j]
                    e.copy(
                        out=AP(
                            tout,
                            b * _RW + r0 * _WC,
                            [[_FO, _P], [_WC, nr], [1, _WC]],
                        ),
                        in_=AP(
                            tin,
                            b * _SEG + r0 * _C,
                            [[_FI, _P], [_C, nr], [1, _WC]],
                        ),
                    ).then_inc(ea, 1)
                for j in range(1, _NCH):
                    wait_chunk(e, b, j)
                    store_chunk(e, b, j, 1)

    _nc_cache = nc
    return nc


def _make_in_maps(x: np.ndarray) -> list[dict]:
    """x: [B, T, C] float32 -> per-core padded bf16 flat inputs."""
    xb = np.asarray(x, dtype=np.float32).astype(_NPBF16)
    xpad = np.zeros((_B, _NP), _NPBF16)
    xpad[:, _PAD : _PAD + _T * _C] = xb.reshape(_B, _T * _C)
    return [
        {"xp": np.ascontiguousarray(xpad[i * _BPC : (i + 1) * _BPC])}
        for i in range(_NCORES)
    ]


def _gather_out(results) -> np.ndarray:
    return np.concatenate(
        [np.asarray(r["out"]).astype(np.float32)[:, :_T, :] for r in results],
        axis=0,
    ).reshape(_B, _T, _WC)


def kernel(x: np.ndarray) -> np.ndarray:
    assert np.asarray(x).shape == (_B, _T, _C)
    nc = _build()
    res = run_bass_kernel_spmd(nc, _make_in_maps(x), list(range(_NCORES)))
    return _gather_out(res.results)
